# revision 7
# baseline (speedup 1.0000x reference)
"""Multi-head causal attention (B=1, S=4096, D=768, H=12) on 8 trn2 NeuronCores.

Sharding: tensor-parallel over heads + causal-balanced split of the query range.
  - cores 0-5 ("late"):  2 heads each, q in [1792, 4096), k in [0, 4096)
  - cores 6-7 ("early"): 6 heads each, q in [0, 1792),  k in [0, 1792)
Each core computes qkv projections for its heads, flash-style causal
softmax(QK^T)V in a transposed layout (seq on the free axis), and a partial
out-projection (contraction over its heads' dims).  The host sums partials,
adds b_out, and transposes back.

All inputs are taken at full shape; slicing/transposition happens on host.
"""

import os
import sys
import threading

sys.path.insert(0, "/opt/trn_rl_repo")

import numpy as np

import concourse.bass as bass
import concourse.mybir as mybir
import concourse.tile as tile
from concourse import bacc
from concourse.masks import make_identity

# ---------------------------------------------------------------- constants
B, S, D, H, DH = 1, 4096, 768, 12, 64
SCALE = DH ** -0.5
P = 128          # sbuf partitions
QT = 256         # query tile (free axis of scores)
KT = 128         # key tile (partition axis of scores)
SPLIT = 1792     # early/late query split point
DT = mybir.dt.float32
DTM = mybir.dt.float32r  # matmul operand dtype (fast fp32 mode)

CLASSES = {
    # name: (n_pairs, q0, q1, k_len)
    "late": (1, SPLIT, S, S),
    "early": (3, 0, SPLIT, SPLIT),
}

def _r(ap):
    return ap


def _groups(n):
    """Split n (even) non-diagonal ktiles into chunks of 3 and 2."""
    out = []
    while n >= 5 or n == 3:
        out.append(3)
        n -= 3
    while n > 0:
        out.append(2)
        n -= 2
    return out


def build_module(cls):
    n_pairs, q0, q1, k_len = CLASSES[cls]
    f_c = 128 * n_pairs          # per-core feature width of each projection
    q_len = q1 - q0
    n_kt_full = k_len // KT      # ktiles of the core's k-support
    n_qt = q_len // QT           # qtiles of the core's q-range
    n_dt = D // P                # 6 contraction tiles for the projections

    nc = bacc.Bacc("TRN2", target_bir_lowering=False, debug=False,
                   enable_asserts=True, num_devices=1)

    xT = nc.dram_tensor("xT", [D, k_len], DTM, kind="ExternalInput")
    wqT = nc.dram_tensor("wqT", [D, f_c], DTM, kind="ExternalInput")
    wkT = nc.dram_tensor("wkT", [D, f_c], DTM, kind="ExternalInput")
    wvT = nc.dram_tensor("wvT", [D, f_c], DTM, kind="ExternalInput")
    bq = nc.dram_tensor("bq", [n_pairs * P, 1], DT, kind="ExternalInput")
    bvb = nc.dram_tensor("bvb", [P, f_c], DT, kind="ExternalInput")
    woT = nc.dram_tensor("woT", [f_c, D], DTM, kind="ExternalInput")
    dmask = nc.dram_tensor("dmask", [P, 2 * QT], DTM, kind="ExternalInput")
    yT = nc.dram_tensor("yT", [D, q_len], DT, kind="ExternalOutput")

    with tile.TileContext(nc) as tc:
        with (
            tc.tile_pool(name="w", bufs=1) as sb_w,
            tc.tile_pool(name="x", bufs=2) as sb_x,
            tc.tile_pool(name="persist", bufs=1) as sb_per,
            tc.tile_pool(name="exp", bufs=3) as sb_exp,
            tc.tile_pool(name="small", bufs=6) as sb_sm,
            tc.tile_pool(name="yout", bufs=3) as sb_y,
            tc.tile_pool(name="ps", bufs=2, space="PSUM") as ps_main,
            tc.tile_pool(name="psO", bufs=2, space="PSUM") as ps_out,
        ):
            # ---------------- constants / weights to SBUF
            wq_sb = sb_w.tile([P, n_dt, f_c], DTM, tag="wq")
            nc.sync.dma_start(out=wq_sb, in_=wqT.rearrange("(t p) f -> p t f", p=P))
            wk_sb = sb_w.tile([P, n_dt, f_c], DTM, tag="wk")
            nc.sync.dma_start(out=wk_sb, in_=wkT.rearrange("(t p) f -> p t f", p=P))
            wv_sb = sb_w.tile([P, n_dt, f_c], DTM, tag="wv")
            nc.sync.dma_start(out=wv_sb, in_=wvT.rearrange("(t p) f -> p t f", p=P))
            bq_sb = sb_w.tile([P, n_pairs], DT, tag="bq")
            nc.sync.dma_start(out=bq_sb, in_=bq.rearrange("(n p) o -> p (n o)", p=P))
            bvb_sb = sb_w.tile([P, f_c], DT, tag="bvb")
            nc.sync.dma_start(out=bvb_sb, in_=bvb.ap())
            wo_sb = sb_w.tile([P, n_pairs, n_dt, P], DTM, tag="wo")
            nc.sync.dma_start(
                out=wo_sb,
                in_=woT.rearrange("(n p) (t m) -> p n t m", p=P, m=P))
            dmask_sb = sb_w.tile([P, 2 * QT], DTM, tag="dmask")
            nc.sync.dma_start(out=dmask_sb, in_=dmask.ap())
            ones2 = sb_w.tile([P, 2], DT, tag="ones2")
            nc.vector.memset(ones2, 1.0)
            ident_f = sb_w.tile([P, P], DT, tag="ident_f")
            make_identity(nc, ident_f)
            ident = sb_w.tile([P, P], DTM, tag="ident")
            nc.vector.tensor_copy(ident, ident_f)

            # ---------------- persistent activations
            qT = [sb_per.tile([P, q_len], DTM, name=f"qT{p}", tag=f"qT{p}")
                  for p in range(n_pairs)]
            kT = [sb_per.tile([P, k_len], DTM, name=f"kT{p}", tag=f"kT{p}")
                  for p in range(n_pairs)]
            vT = [sb_per.tile([P, k_len], DTM, name=f"vT{p}", tag=f"vT{p}")
                  for p in range(n_pairs)]
            # per ktile: [V_A | 1 | V_B | 1] with k on partitions
            vkt = [[sb_per.tile([P, 130], DTM, name=f"v{p}_{k}", tag=f"v{p}_{k}")
                    for k in range(n_kt_full)] for p in range(n_pairs)]

            # ---------------- phase 1: projections  (qkvT = W^T-slices @ xT)
            # s-chunks of up to 512 over the k-support
            chunks = []
            s0 = 0
            while s0 < k_len:
                w = min(512, k_len - s0)
                chunks.append((s0, w))
                s0 += w
            for (s0, w) in chunks:
                xts = []
                for dti in range(n_dt):
                    xt = sb_x.tile([P, 512], DTM, tag=f"xt{dti}")
                    nc.sync.dma_start(
                        out=xt[:, :w],
                        in_=xT[dti * P:(dti + 1) * P, s0:s0 + w])
                    xts.append(xt)
                for p in range(n_pairs):
                    ps = ps_main.tile([P, 3, 512], DT, tag="ps")
                    do_q = s0 + w > q0  # chunk overlaps the q-range
                    for dti in range(n_dt):
                        first, last = dti == 0, dti == n_dt - 1
                        if do_q:
                            nc.tensor.matmul(
                                ps[:, 0, :w],
                                _r(wq_sb[:, dti, p * P:(p + 1) * P]),
                                _r(xts[dti][:, :w]), start=first, stop=last)
                        nc.tensor.matmul(
                            ps[:, 1, :w],
                            _r(wk_sb[:, dti, p * P:(p + 1) * P]),
                            _r(xts[dti][:, :w]), start=first, stop=last)
                        nc.tensor.matmul(
                            ps[:, 2, :w],
                            _r(wv_sb[:, dti, p * P:(p + 1) * P]),
                            _r(xts[dti][:, :w]), start=first, stop=last)
                    if do_q:  # q += bias, into persistent qT (q-range cols)
                        lo = max(s0, q0)
                        nc.vector.tensor_scalar_add(
                            qT[p][:, lo - q0:s0 + w - q0],
                            ps[:, 0, lo - s0:w], bq_sb[:, p:p + 1])
                    nc.vector.tensor_copy(kT[p][:, s0:s0 + w], ps[:, 1, :w])
                    nc.vector.tensor_copy(vT[p][:, s0:s0 + w], ps[:, 2, :w])

            # V: transpose to [k, dh] tiles and add bias; append ones column
            for p in range(n_pairs):
                for k in range(n_kt_full):
                    pt = ps_main.tile([P, P], DTM, tag="ps")
                    nc.tensor.transpose(
                        pt, vT[p][:, k * KT:(k + 1) * KT], ident)
                    vt = vkt[p][k]
                    nc.vector.tensor_add(
                        vt[:, 0:64], pt[:, 0:64], bvb_sb[:, p * P:p * P + 64])
                    nc.vector.tensor_add(
                        vt[:, 65:129], pt[:, 64:128],
                        bvb_sb[:, p * P + 64:(p + 1) * P])
                    nc.vector.tensor_copy(
                        vt.rearrange("p (h c) -> p h c", h=2)[:, :, 64], ones2)

            # ---------------- phase 2: attention + out-projection
            for qt in range(n_qt):
                g = q0 // QT + qt          # global strip index
                n_kt = 2 * g + 2           # causal ktiles for this strip
                a_tiles = []
                for p in range(n_pairs):
                    outA = ps_out.tile([65, QT], DT, tag="out")
                    outB = ps_out.tile([65, QT], DT, tag="out")
                    qA = qT[p][0:64, qt * QT:(qt + 1) * QT]
                    qB = qT[p][64:128, qt * QT:(qt + 1) * QT]

                    kt_done = 0
                    plan = [(c, False) for c in _groups(n_kt - 2)] + [(2, True)]
                    for (gsz, diag) in plan:
                        kts = list(range(kt_done, kt_done + gsz))
                        ps_sc = ps_main.tile([P, 2, 3, QT], DT, tag="ps")
                        for hi, qh in ((0, qA), (1, qB)):
                            for j, k in enumerate(kts):
                                nc.tensor.matmul(
                                    ps_sc[:, hi, j, :],
                                    _r(kT[p][hi * 64:hi * 64 + 64,
                                             k * KT:(k + 1) * KT]),
                                    _r(qh), start=True, stop=True)
                        ex = sb_exp.tile([P, 2, 3, QT], DTM, tag="ex")
                        for hi in (0, 1):
                            nc.scalar.activation(
                                ex[:, hi, 0:gsz, :], ps_sc[:, hi, 0:gsz, :],
                                mybir.ActivationFunctionType.Exp, scale=SCALE)
                        if diag:
                            for hi in (0, 1):
                                nc.vector.tensor_mul(
                                    ex[:, hi, 0:2, :], ex[:, hi, 0:2, :],
                                    dmask_sb.rearrange("p (a q) -> p a q", a=2))
                        for hi, outX in ((0, outA), (1, outB)):
                            for j, k in enumerate(kts):
                                nc.tensor.matmul(
                                    outX,
                                    _r(vkt[p][k][:, hi * 65:hi * 65 + 65]),
                                    _r(ex[:, hi, j, :]),
                                    start=(k == 0), stop=(k == n_kt - 1))
                        kt_done += gsz

                    # normalize: a = num / den  (den broadcast over partitions)
                    aT = sb_sm.tile([P, QT], DTM, tag="aT")
                    for hi, outX in ((0, outA), (1, outB)):
                        rr = sb_sm.tile([1, QT], DT, tag="rr")
                        nc.vector.reciprocal(rr, outX[64:65, :])
                        rb = sb_sm.tile([64, QT], DT, tag="rb")
                        nc.gpsimd.partition_broadcast(rb, rr)
                        nc.vector.tensor_mul(
                            aT[hi * 64:(hi + 1) * 64, :], outX[0:64, :], rb)
                    a_tiles.append(aT)

                # partial out-projection for this qtile
                for mt in range(n_dt):
                    ps_y = ps_main.tile([P, QT], DT, tag="ps")
                    for p in range(n_pairs):
                        nc.tensor.matmul(
                            ps_y, _r(wo_sb[:, p, mt, :]), _r(a_tiles[p]),
                            start=(p == 0), stop=(p == n_pairs - 1))
                    ysb = sb_y.tile([P, QT], DT, tag="y")
                    nc.vector.tensor_copy(ysb, ps_y)
                    nc.sync.dma_start(
                        out=yT[mt * P:(mt + 1) * P, qt * QT:(qt + 1) * QT],
                        in_=ysb)

    nc.compile()
    return nc


# ---------------------------------------------------------------- host side
def _head_cols(heads):
    """column indices into a [*, 768] head-blocked axis for the given heads"""
    return np.concatenate([np.arange(h * DH, (h + 1) * DH) for h in heads])


def make_in_maps(x, W_in, b_in, W_out):
    """Returns (late_in_maps[6], early_in_maps[2])."""
    xT = np.ascontiguousarray(x.reshape(S, D).T)          # [768, 4096]
    WT = np.ascontiguousarray(W_in.T)                     # [768, 2304]
    WoT = np.ascontiguousarray(W_out.T)                   # [768, 768]

    tri = np.triu(np.ones((P, P), np.float32))            # k <= q
    dm = np.zeros((P, 2 * QT), np.float32)
    dm[:, 0:128] = tri
    dm[:, 128:256] = 1.0
    dm[:, 256:384] = 0.0
    dm[:, 384:512] = tri

    def core_inputs(heads, cls):
        _, q0, q1, k_len = CLASSES[cls]
        cols = _head_cols(heads)
        wq = np.ascontiguousarray(WT[:, cols])
        wk = np.ascontiguousarray(WT[:, 768 + cols])
        wv = np.ascontiguousarray(WT[:, 1536 + cols])
        bqc = np.ascontiguousarray(b_in[cols][:, None])
        bvc = np.ascontiguousarray(
            np.broadcast_to(b_in[1536 + cols][None, :], (P, len(cols))))
        wo = np.ascontiguousarray(WoT[cols, :])
        return {
            "xT": np.ascontiguousarray(xT[:, :k_len]),
            "wqT": wq, "wkT": wk, "wvT": wv,
            "bq": bqc, "bvb": bvc, "woT": wo, "dmask": dm,
        }

    late = [core_inputs([2 * c, 2 * c + 1], "late") for c in range(6)]
    early = [core_inputs(list(range(6 * e, 6 * e + 6)), "early")
             for e in range(2)]
    return late, early


def assemble_output(late_res, early_res, b_out):
    yT = np.zeros((D, S), np.float32)
    for r in late_res:
        yT[:, SPLIT:] += r["yT"]
    for r in early_res:
        yT[:, :SPLIT] += r["yT"]
    y = yT.T + b_out[None, :]
    return y.reshape(B, S, D).astype(np.float32)


# ------------------------------------------------- pjrt runner (explicit devices)
def _run_group(nc, in_maps, devices):
    """run_bass_via_pjrt equivalent on an explicit device subset."""
    import jax
    from jax.sharding import Mesh, PartitionSpec
    from jax.experimental.shard_map import shard_map
    from concourse import bass2jax
    from concourse.bass2jax import _bass_exec_p, partition_id_tensor

    bass2jax.install_neuronx_cc_hook()
    n_cores = len(in_maps)
    partition_name = (nc.partition_id_tensor.name
                      if nc.partition_id_tensor else None)

    in_names, out_names, out_avals, zero_outs = [], [], [], []
    for alloc in nc.m.functions[0].allocations:
        if not isinstance(alloc, mybir.MemoryLocationSet):
            continue
        name = alloc.memorylocations[0].name
        if alloc.kind == "ExternalInput":
            if name != partition_name:
                in_names.append(name)
        elif alloc.kind == "ExternalOutput":
            shape = tuple(alloc.tensor_shape)
            dtype = mybir.dt.np(alloc.dtype)
            out_names.append(name)
            out_avals.append(jax.core.ShapedArray(shape, dtype))
            zero_outs.append(np.zeros(shape, dtype))
    n_params = len(in_names)
    n_outs = len(out_avals)
    in_names = in_names + out_names
    if partition_name is not None:
        in_names.append(partition_name)
    donate = tuple(range(n_params, n_params + n_outs))

    def _body(*args):
        operands = list(args)
        if partition_name is not None:
            operands.append(partition_id_tensor())
        outs = _bass_exec_p.bind(
            *operands,
            out_avals=tuple(out_avals),
            in_names=tuple(in_names),
            out_names=tuple(out_names),
            lowering_input_output_aliases=(),
            sim_require_finite=True,
            sim_require_nnan=True,
            nc=nc,
        )
        return tuple(outs)

    per_core = [[np.asarray(m[name]) for name in in_names[:n_params]]
                for m in in_maps]
    if n_cores == 1:
        out_arrs = jax.jit(_body, donate_argnums=donate, keep_unused=True)(
            *per_core[0], *zero_outs)
        return [{n: np.asarray(out_arrs[i]) for i, n in enumerate(out_names)}]

    mesh = Mesh(np.asarray(devices), ("core",))
    in_specs = (PartitionSpec("core"),) * (n_params + n_outs)
    out_specs = (PartitionSpec("core"),) * len(out_names)
    sharded = jax.jit(
        shard_map(_body, mesh=mesh, in_specs=in_specs, out_specs=out_specs,
                  check_rep=False),
        donate_argnums=donate, keep_unused=True)
    concat_in = [np.concatenate([per_core[c][i] for c in range(n_cores)],
                                axis=0) for i in range(n_params)]
    concat_zeros = [np.zeros((n_cores * z.shape[0], *z.shape[1:]), z.dtype)
                    for z in zero_outs]
    out_arrs = sharded(*concat_in, *concat_zeros)
    return [
        {n: np.asarray(out_arrs[i]).reshape(n_cores, *out_avals[i].shape)[c]
         for i, n in enumerate(out_names)}
        for c in range(n_cores)
    ]


_MODULES = {}


def _get_module(cls):
    if cls not in _MODULES:
        _MODULES[cls] = build_module(cls)
    return _MODULES[cls]


def kernel(x, W_in, b_in, W_out, b_out):
    import jax
    x = np.asarray(x, np.float32)
    W_in = np.asarray(W_in, np.float32)
    b_in = np.asarray(b_in, np.float32)
    W_out = np.asarray(W_out, np.float32)
    b_out = np.asarray(b_out, np.float32)

    late_maps, early_maps = make_in_maps(x, W_in, b_in, W_out)
    nc_late = _get_module("late")
    nc_early = _get_module("early")

    devs = jax.devices()
    results = {}
    errs = {}

    def run(tag, nc, maps, devices):
        try:
            results[tag] = _run_group(nc, maps, devices)
        except Exception as e:  # noqa: BLE001
            errs[tag] = e

    # compile sequentially (first call traces+compiles), then the cached
    # executables run; threads let the two device groups execute concurrently
    t1 = threading.Thread(target=run, args=("late", nc_late, late_maps, devs[0:6]))
    t2 = threading.Thread(target=run, args=("early", nc_early, early_maps, devs[6:8]))
    t1.start()
    t1.join()
    t2.start()
    t2.join()
    if errs:
        raise next(iter(errs.values()))

    return assemble_output(results["late"], results["early"], b_out)


# revision 16
# speedup vs baseline: 1.0050x; 1.0050x over previous
"""Multi-head causal attention (B=1, S=4096, D=768, H=12) on 8 trn2 NeuronCores.

Sharding: tensor-parallel over heads + causal-balanced split of the query range.
  - cores 0-5 ("late"):  2 heads each, q in [1792, 4096), k in [0, 4096)
  - cores 6-7 ("early"): 6 heads each, q in [0, 1792),  k in [0, 1792)
Each core computes qkv projections for its heads, flash-style causal
softmax(QK^T)V in a transposed layout (seq on the free axis), and a partial
out-projection (contraction over its heads' dims).  The host sums partials,
adds b_out, and transposes back.

All inputs are taken at full shape; slicing/transposition happens on host.
"""

import os
import sys
import threading

sys.path.insert(0, "/opt/trn_rl_repo")

import numpy as np
import ml_dtypes

import concourse.bass as bass
import concourse.mybir as mybir
import concourse.tile as tile
from concourse import bacc
from concourse.masks import make_identity

# ---------------------------------------------------------------- constants
B, S, D, H, DH = 1, 4096, 768, 12, 64
SCALE = DH ** -0.5
P = 128          # sbuf partitions
QT = 256         # query tile (free axis of scores)
KT = 128         # key tile (partition axis of scores)
SPLIT = 1792     # early/late query split point
DT = mybir.dt.float32
DTM = mybir.dt.float32r  # qkv/projection operand dtype (fast fp32 mode)
DTB = mybir.dt.bfloat16   # attention inner-chain dtype (expT, V, aT, W_out)

CLASSES = {
    # name: (n_pairs, q0, q1, k_len)
    "late": (1, SPLIT, S, S),
    "early": (3, 0, SPLIT, SPLIT),
}

def _r(ap):
    return ap


def _groups(n):
    """Split n (even) non-diagonal ktiles into chunks of 3 and 2."""
    out = []
    while n >= 5 or n == 3:
        out.append(3)
        n -= 3
    while n > 0:
        out.append(2)
        n -= 2
    return out


def build_module(cls):
    n_pairs, q0, q1, k_len = CLASSES[cls]
    f_c = 128 * n_pairs          # per-core feature width of each projection
    q_len = q1 - q0
    n_kt_full = k_len // KT      # ktiles of the core's k-support
    n_qt = q_len // QT           # qtiles of the core's q-range
    n_dt = D // P                # 6 contraction tiles for the projections

    nc = bacc.Bacc("TRN2", target_bir_lowering=False, debug=False,
                   enable_asserts=True, num_devices=1)

    xT = nc.dram_tensor("xT", [D, k_len], DTM, kind="ExternalInput")
    wqT = nc.dram_tensor("wqT", [D, f_c], DTM, kind="ExternalInput")
    wkT = nc.dram_tensor("wkT", [D, f_c], DTM, kind="ExternalInput")
    wvT = nc.dram_tensor("wvT", [D, f_c], DTM, kind="ExternalInput")
    bq = nc.dram_tensor("bq", [n_pairs * P, 1], DT, kind="ExternalInput")
    bvb = nc.dram_tensor("bvb", [P, f_c], DT, kind="ExternalInput")
    woT = nc.dram_tensor("woT", [f_c, D], DTM, kind="ExternalInput")
    dmask = nc.dram_tensor("dmask", [P, 2, 2, QT], DTB, kind="ExternalInput")
    yT = nc.dram_tensor("yT", [D, q_len], DT, kind="ExternalOutput")

    KB0 = os.environ.get("KBISECT", "full")
    DTC = DTM if KB0 == "exf32r" else DTB   # attention-chain dtype
    with tile.TileContext(nc) as tc:
        with (
            tc.tile_pool(name="w", bufs=1) as sb_w,
            tc.tile_pool(name="x", bufs=2) as sb_x,
            tc.tile_pool(name="persist", bufs=1) as sb_per,
            tc.tile_pool(name="exp", bufs=3) as sb_exp,
            tc.tile_pool(name="small", bufs=6) as sb_sm,
            tc.tile_pool(name="yout", bufs=3) as sb_y,
            tc.tile_pool(name="ps", bufs=2, space="PSUM") as ps_main,
            tc.tile_pool(name="psO", bufs=2, space="PSUM") as ps_out,
        ):
            # ---------------- constants / weights to SBUF
            wq_sb = sb_w.tile([P, n_dt, f_c], DTM, tag="wq")
            nc.sync.dma_start(out=wq_sb, in_=wqT.rearrange("(t p) f -> p t f", p=P))
            wk_sb = sb_w.tile([P, n_dt, f_c], DTM, tag="wk")
            nc.sync.dma_start(out=wk_sb, in_=wkT.rearrange("(t p) f -> p t f", p=P))
            wv_sb = sb_w.tile([P, n_dt, f_c], DTM, tag="wv")
            nc.sync.dma_start(out=wv_sb, in_=wvT.rearrange("(t p) f -> p t f", p=P))
            bq_sb = sb_w.tile([P, n_pairs], DT, tag="bq")
            nc.sync.dma_start(out=bq_sb, in_=bq.rearrange("(n p) o -> p (n o)", p=P))
            bvb_sb = sb_w.tile([P, f_c], DT, tag="bvb")
            nc.sync.dma_start(out=bvb_sb, in_=bvb.ap())
            wo_sb = sb_w.tile([P, n_pairs, n_dt, P], DTM, tag="wo")
            nc.sync.dma_start(
                out=wo_sb,
                in_=woT.rearrange("(n p) (t m) -> p n t m", p=P, m=P))
            dmask_sb = sb_w.tile([P, 2, 2, QT], DTB, tag="dmask")
            nc.sync.dma_start(out=dmask_sb, in_=dmask.ap())
            ones2 = sb_w.tile([P, 2], DT, tag="ones2")
            nc.vector.memset(ones2, 1.0)
            ident_f = sb_w.tile([P, P], DT, tag="ident_f")
            make_identity(nc, ident_f)
            ident = sb_w.tile([P, P], DTM, tag="ident")
            nc.vector.tensor_copy(ident, ident_f)

            # ---------------- persistent activations
            qT = [sb_per.tile([P, q_len], DTM, name=f"qT{p}", tag=f"qT{p}")
                  for p in range(n_pairs)]
            kT = [sb_per.tile([P, k_len], DTM, name=f"kT{p}", tag=f"kT{p}")
                  for p in range(n_pairs)]
            vT = [sb_per.tile([P, k_len], DTM, name=f"vT{p}", tag=f"vT{p}")
                  for p in range(n_pairs)]
            # per ktile: [V_A | 1 | V_B | 1] with k on partitions
            vkt = [[sb_per.tile([P, 132], DTC, name=f"v{p}_{k}", tag=f"v{p}_{k}")
                    for k in range(n_kt_full)] for p in range(n_pairs)]

            # ---------------- phase 1: projections  (qkvT = W^T-slices @ xT)
            # s-chunks of up to 512 over the k-support
            chunks = []
            s0 = 0
            while s0 < k_len:
                w = min(512, k_len - s0)
                chunks.append((s0, w))
                s0 += w
            for (s0, w) in chunks:
                xts = []
                for dti in range(n_dt):
                    xt = sb_x.tile([P, 512], DTM, tag=f"xt{dti}")
                    nc.sync.dma_start(
                        out=xt[:, :w],
                        in_=xT[dti * P:(dti + 1) * P, s0:s0 + w])
                    xts.append(xt)
                for p in range(n_pairs):
                    ps = ps_main.tile([P, 3, 512], DT, tag="ps")
                    do_q = s0 + w > q0  # chunk overlaps the q-range
                    for dti in range(n_dt):
                        first, last = dti == 0, dti == n_dt - 1
                        if do_q:
                            nc.tensor.matmul(
                                ps[:, 0, :w],
                                _r(wq_sb[:, dti, p * P:(p + 1) * P]),
                                _r(xts[dti][:, :w]), start=first, stop=last)
                        nc.tensor.matmul(
                            ps[:, 1, :w],
                            _r(wk_sb[:, dti, p * P:(p + 1) * P]),
                            _r(xts[dti][:, :w]), start=first, stop=last)
                        nc.tensor.matmul(
                            ps[:, 2, :w],
                            _r(wv_sb[:, dti, p * P:(p + 1) * P]),
                            _r(xts[dti][:, :w]), start=first, stop=last)
                    if do_q:  # q += bias, into persistent qT (q-range cols)
                        lo = max(s0, q0)
                        nc.vector.tensor_scalar_add(
                            qT[p][:, lo - q0:s0 + w - q0],
                            ps[:, 0, lo - s0:w], bq_sb[:, p:p + 1])
                    nc.vector.tensor_copy(kT[p][:, s0:s0 + w], ps[:, 1, :w])
                    nc.vector.tensor_copy(vT[p][:, s0:s0 + w], ps[:, 2, :w])

            # V: transpose to [k, dh] tiles and add bias; append ones column
            for p in range(n_pairs):
                for k in range(n_kt_full):
                    pt = ps_main.tile([P, P], DTM, tag="ps")
                    nc.tensor.transpose(
                        pt, vT[p][:, k * KT:(k + 1) * KT], ident)
                    vt = vkt[p][k]
                    nc.vector.tensor_add(
                        vt[:, 0:64], pt[:, 0:64], bvb_sb[:, p * P:p * P + 64])
                    nc.vector.tensor_add(
                        vt[:, 66:130], pt[:, 64:128],
                        bvb_sb[:, p * P + 64:(p + 1) * P])
                    nc.vector.tensor_copy(
                        vt.rearrange("p (h c) -> p h c", h=2)[:, :, 64], ones2)

            # ---------------- phase 2: attention + out-projection
            KB = os.environ.get("KBISECT", "full")
            if KB == "noattn":
                for qt2 in range(n_qt):
                    ysb0 = sb_y.tile([P, QT], DT, tag="y")
                    nc.vector.tensor_copy(ysb0, qT[0].bitcast(DT)[:, qt2 * QT:(qt2 + 1) * QT])
                    for mt2 in range(n_dt):
                        nc.sync.dma_start(
                            out=yT[mt2 * P:(mt2 + 1) * P, qt2 * QT:(qt2 + 1) * QT],
                            in_=ysb0)
                n_qt_eff = 0
            else:
                n_qt_eff = n_qt
            for qt in range(n_qt_eff):
                g = q0 // QT + qt          # global strip index
                n_kt = 2 * g + 2           # causal ktiles for this strip
                a_tiles = []
                for p in range(n_pairs):
                    outA = ps_out.tile([65, QT], DT, tag="out")
                    outB = ps_out.tile([65, QT], DT, tag="out")
                    qA = qT[p][0:64, qt * QT:(qt + 1) * QT]
                    qB = qT[p][64:128, qt * QT:(qt + 1) * QT]

                    kt_done = 0
                    plan = [(c, False) for c in _groups(n_kt - 2)] + [(2, True)]
                    for (gsz, diag) in plan:
                        kts = list(range(kt_done, kt_done + gsz))
                        ps_sc = ps_main.tile([P, 2, 3, QT], DT, tag="ps")
                        for hi, qh in ((0, qA), (1, qB)):
                            for j, k in enumerate(kts):
                                nc.tensor.matmul(
                                    ps_sc[:, hi, j, :],
                                    kT[p][hi * 64:hi * 64 + 64,
                                          k * KT:(k + 1) * KT],
                                    qh, start=True, stop=True)
                        ex = sb_exp.tile([P, 2, 3, QT], DTC, tag="ex")
                        for hi in (0, 1):
                            nc.scalar.activation(
                                ex[:, hi, 0:gsz, :], ps_sc[:, hi, 0:gsz, :],
                                mybir.ActivationFunctionType.Exp, scale=SCALE)
                        if diag and KB != "nomask":
                            for hi in (0, 1):
                                nc.vector.tensor_mul(
                                    ex[:, hi, 0:2, :], ex[:, hi, 0:2, :],
                                    dmask_sb[:, :, 0, :])
                        for hi, outX in ((0, outA), (1, outB)):
                            for j, k in enumerate(kts):
                                nc.tensor.matmul(
                                    outX,
                                    vkt[p][k][:, hi * 66:hi * 66 + 65],
                                    ex[:, hi, j, :],
                                    start=(k == 0), stop=(k == n_kt - 1))
                        kt_done += gsz

                    # normalize: a = num / den  (den broadcast over partitions)
                    aT = sb_sm.tile([P, QT], DTM, tag="aT")
                    for hi, outX in ((0, outA), (1, outB)):
                        rr = sb_sm.tile([1, QT], DT, tag="rr")
                        nc.vector.reciprocal(rr, outX[64:65, :])
                        rb = sb_sm.tile([64, QT], DT, tag="rb")
                        nc.gpsimd.partition_broadcast(rb, rr)
                        nc.vector.tensor_mul(
                            aT[hi * 64:(hi + 1) * 64, :], outX[0:64, :], rb)
                    a_tiles.append(aT)

                # partial out-projection for this qtile
                for mt in range(n_dt):
                    ps_y = ps_main.tile([P, QT], DT, tag="ps")
                    for p in range(n_pairs):
                        nc.tensor.matmul(
                            ps_y, wo_sb[:, p, mt, :], a_tiles[p],
                            start=(p == 0), stop=(p == n_pairs - 1))
                    ysb = sb_y.tile([P, QT], DT, tag="y")
                    nc.vector.tensor_copy(ysb, ps_y)
                    nc.sync.dma_start(
                        out=yT[mt * P:(mt + 1) * P, qt * QT:(qt + 1) * QT],
                        in_=ysb)

    nc.compile()
    return nc


# ---------------------------------------------------------------- host side
def _head_cols(heads):
    """column indices into a [*, 768] head-blocked axis for the given heads"""
    return np.concatenate([np.arange(h * DH, (h + 1) * DH) for h in heads])


def make_in_maps(x, W_in, b_in, W_out):
    """Returns (late_in_maps[6], early_in_maps[2])."""
    xT = np.ascontiguousarray(x.reshape(S, D).T)          # [768, 4096]
    WT = np.ascontiguousarray(W_in.T)                     # [768, 2304]
    WoT = np.ascontiguousarray(W_out.T)                   # [768, 768]

    tri = np.triu(np.ones((P, P), np.float32))            # k <= q
    dm0 = np.zeros((P, QT), np.float32)                   # diag ktile j=0
    dm0[:, 0:128] = tri
    dm0[:, 128:256] = 1.0
    dm1 = np.zeros((P, QT), np.float32)                   # diag ktile j=1
    dm1[:, 128:256] = tri
    dm = np.stack([dm0, dm0, dm1, dm1], axis=1)           # [P, (j hi), QT]
    dm = dm.reshape(P, 2, 2, QT).astype(ml_dtypes.bfloat16)

    def core_inputs(heads, cls):
        _, q0, q1, k_len = CLASSES[cls]
        cols = _head_cols(heads)
        wq = np.ascontiguousarray(WT[:, cols])
        wk = np.ascontiguousarray(WT[:, 768 + cols])
        wv = np.ascontiguousarray(WT[:, 1536 + cols])
        bqc = np.ascontiguousarray(b_in[cols][:, None])
        bvc = np.ascontiguousarray(
            np.broadcast_to(b_in[1536 + cols][None, :], (P, len(cols))))
        wo = np.ascontiguousarray(WoT[cols, :])
        return {
            "xT": np.ascontiguousarray(xT[:, :k_len]),
            "wqT": wq, "wkT": wk, "wvT": wv,
            "bq": bqc, "bvb": bvc, "woT": wo, "dmask": dm,
        }

    late = [core_inputs([2 * c, 2 * c + 1], "late") for c in range(6)]
    early = [core_inputs(list(range(6 * e, 6 * e + 6)), "early")
             for e in range(2)]
    return late, early


def assemble_output(late_res, early_res, b_out):
    yT = np.zeros((D, S), np.float32)
    for r in late_res:
        yT[:, SPLIT:] += r["yT"]
    for r in early_res:
        yT[:, :SPLIT] += r["yT"]
    y = yT.T + b_out[None, :]
    return y.reshape(B, S, D).astype(np.float32)


# ------------------------------------------------- pjrt runner (explicit devices)
def _run_group(nc, in_maps, devices):
    """run_bass_via_pjrt equivalent on an explicit device subset."""
    import jax
    from jax.sharding import Mesh, PartitionSpec
    from jax.experimental.shard_map import shard_map
    from concourse import bass2jax
    from concourse.bass2jax import _bass_exec_p, partition_id_tensor

    bass2jax.install_neuronx_cc_hook()
    n_cores = len(in_maps)
    partition_name = (nc.partition_id_tensor.name
                      if nc.partition_id_tensor else None)

    in_names, out_names, out_avals, zero_outs = [], [], [], []
    for alloc in nc.m.functions[0].allocations:
        if not isinstance(alloc, mybir.MemoryLocationSet):
            continue
        name = alloc.memorylocations[0].name
        if alloc.kind == "ExternalInput":
            if name != partition_name:
                in_names.append(name)
        elif alloc.kind == "ExternalOutput":
            shape = tuple(alloc.tensor_shape)
            dtype = mybir.dt.np(alloc.dtype)
            out_names.append(name)
            out_avals.append(jax.core.ShapedArray(shape, dtype))
            zero_outs.append(np.zeros(shape, dtype))
    n_params = len(in_names)
    n_outs = len(out_avals)
    in_names = in_names + out_names
    if partition_name is not None:
        in_names.append(partition_name)
    donate = tuple(range(n_params, n_params + n_outs))

    def _body(*args):
        operands = list(args)
        if partition_name is not None:
            operands.append(partition_id_tensor())
        outs = _bass_exec_p.bind(
            *operands,
            out_avals=tuple(out_avals),
            in_names=tuple(in_names),
            out_names=tuple(out_names),
            lowering_input_output_aliases=(),
            sim_require_finite=True,
            sim_require_nnan=True,
            nc=nc,
        )
        return tuple(outs)

    per_core = [[np.asarray(m[name]) for name in in_names[:n_params]]
                for m in in_maps]
    if n_cores == 1:
        out_arrs = jax.jit(_body, donate_argnums=donate, keep_unused=True)(
            *per_core[0], *zero_outs)
        return [{n: np.asarray(out_arrs[i]) for i, n in enumerate(out_names)}]

    mesh = Mesh(np.asarray(devices), ("core",))
    in_specs = (PartitionSpec("core"),) * (n_params + n_outs)
    out_specs = (PartitionSpec("core"),) * len(out_names)
    sharded = jax.jit(
        shard_map(_body, mesh=mesh, in_specs=in_specs, out_specs=out_specs,
                  check_rep=False),
        donate_argnums=donate, keep_unused=True)
    concat_in = [np.concatenate([per_core[c][i] for c in range(n_cores)],
                                axis=0) for i in range(n_params)]
    concat_zeros = [np.zeros((n_cores * z.shape[0], *z.shape[1:]), z.dtype)
                    for z in zero_outs]
    out_arrs = sharded(*concat_in, *concat_zeros)
    return [
        {n: np.asarray(out_arrs[i]).reshape(n_cores, *out_avals[i].shape)[c]
         for i, n in enumerate(out_names)}
        for c in range(n_cores)
    ]


_MODULES = {}


def _get_module(cls):
    if cls not in _MODULES:
        _MODULES[cls] = build_module(cls)
    return _MODULES[cls]


def kernel(x, W_in, b_in, W_out, b_out):
    import jax
    x = np.asarray(x, np.float32)
    W_in = np.asarray(W_in, np.float32)
    b_in = np.asarray(b_in, np.float32)
    W_out = np.asarray(W_out, np.float32)
    b_out = np.asarray(b_out, np.float32)

    late_maps, early_maps = make_in_maps(x, W_in, b_in, W_out)
    nc_late = _get_module("late")
    nc_early = _get_module("early")

    devs = jax.devices()
    results = {}
    errs = {}

    def run(tag, nc, maps, devices):
        try:
            results[tag] = _run_group(nc, maps, devices)
        except Exception as e:  # noqa: BLE001
            errs[tag] = e

    # compile sequentially (first call traces+compiles), then the cached
    # executables run; threads let the two device groups execute concurrently
    t1 = threading.Thread(target=run, args=("late", nc_late, late_maps, devs[0:6]))
    t2 = threading.Thread(target=run, args=("early", nc_early, early_maps, devs[6:8]))
    t1.start()
    t1.join()
    t2.start()
    t2.join()
    if errs:
        raise next(iter(errs.values()))

    return assemble_output(results["late"], results["early"], b_out)
